# revision 18
# baseline (speedup 1.0000x reference)
"""ConvCaps (routing-by-agreement) Trainium2 kernel, v2.

Problem: pose (4, 512, 32, 32) f32, W (288, 512, 16) f32 ->
         out (4, 512, 15, 15) f32.

Per spatial position l (15x15=225 per batch, 900 total, padded to
8 cores x 128 lanes): votes V[k, bd] = sum_c W[k,bd,c] p[l,k,c],
3 routing iterations (softmax over B=32 output caps, D=16).

v2 design:
  - W+pose resident in SBUF as bf16, row-tiled layout: partition
    32*j + c holds k=4g+j; one DMA, reused for both routing iters.
  - Votes via PE row-tiling: 4 concurrent K=16 matmuls
    (tile_position=(32j,0)) -> ~4x PE throughput.
  - Pass 1 (uniform c) via 128-deep contraction: 36 matmuls.
  - Routing engine split per group g (4 k's, 2048 elems/lane):
      scalar: psum->sbuf bf16 evac + exp
      DVE   : um = vt*w (bf16 2x), u = reduce_d, Z, 1/Z, sg = reduce_g,
              s_acc accumulation
      gpsimd: c = e*rZ, sm = vt*c (split with DVE)
"""

import numpy as np
import ml_dtypes

import concourse.bass as bass
import concourse.tile as tile
from concourse import mybir
from concourse.bass_utils import run_bass_kernel_spmd
from concourse.vector_clock import ScopedClock

# ---- problem constants ----
A, B, K, P, STRIDE, ITERS = 32, 32, 3, 4, 2, 3
C = P * P            # 16
D = P * P            # 16
KK = K * K           # 9
KKA = KK * A         # 288
BD = B * D           # 512
EPS = 1e-8
H = W_IN = 32
OH = (H - K) // STRIDE + 1   # 15
OW = OH                      # 15
L = OH * OW                  # 225
NB = 4                       # batch
NPOS = NB * L                # 900
NCORES = 8
LP = 128                     # positions per core (padded)
NPOS_PAD = NCORES * LP       # 1024

G = 4                        # k-group size (4 row-tiled matmuls)
NG = KKA // G                # 72 groups
KD = 8                       # k's per deep chunk (pass 1)
NDC = KKA // KD              # 36 deep chunks

F32 = mybir.dt.float32
F32R = mybir.dt.float32r
BF16 = mybir.dt.bfloat16

AX = mybir.AxisListType
OP = mybir.AluOpType
ACT = mybir.ActivationFunctionType

BF = ml_dtypes.bfloat16


class _ChunkedDrainTileContext(tile.TileContext):
    """Work around a walrus limit of 2 sem-waits per CTRL instruction:
    split the kernel-tail drain's waits across per-processor drains."""

    def _drain_and_barrier(self, tick_clock, wait_clock):
        vclock = tick_clock.global_clock
        observed = ScopedClock()
        for i in range(len(vclock)):
            if vclock[i] > 0:
                partial = ScopedClock()
                partial.require_at_least(None, i, vclock[i])
                d = self.nc.sync.drain()
                wait_clock.add_sem_waits(d.ins, partial, observed)
                observed.update_past(partial)
        drain_inst = self.nc.sync.drain()
        wait_clock.add_sem_waits(
            drain_inst.ins, ScopedClock({None: tick_clock.global_clock}), observed
        )
        self.nc.all_engine_barrier()
        assert self.sems is not None
        popped = self.nc._tile_sem_poison_stack.pop()
        assert popped is self._sem_poison
        self.nc.clear_and_free_semaphores(list(self.sems.allocated().values()))
        self.nc.all_engine_barrier()


def _bd(t):
    if len(t.shape) == 2:
        return t.rearrange("p (b d) -> p b d", d=D)
    return t.rearrange("p g (b d) -> p g b d", d=D)


def _squash(nc, pool, ss, eps_t):
    """ss: [128, 512] f32 SBUF (already scaled). returns v [128, 512] f32."""
    sq = pool.tile([LP, BD], F32, tag="sq")
    nc.vector.tensor_mul(out=sq, in0=ss, in1=ss)
    n2 = pool.tile([LP, B], F32, tag="n2")
    nc.vector.tensor_reduce(out=n2, in_=_bd(sq), axis=AX.X, op=OP.add)
    # (n2+eps)^-1/2 = exp(-0.5*ln(n2+eps))
    lg = pool.tile([LP, B], F32, tag="lg")
    nc.scalar.activation(out=lg, in_=n2, func=ACT.Ln, bias=eps_t, scale=1.0)
    rs = pool.tile([LP, B], F32, tag="rs")
    nc.scalar.activation(out=rs, in_=lg, func=ACT.Exp, bias=0.0, scale=-0.5)
    np1 = pool.tile([LP, B], F32, tag="np1")
    nc.vector.tensor_scalar_add(out=np1, in0=n2, scalar1=1.0)
    rnp1 = pool.tile([LP, B], F32, tag="rnp1")
    nc.vector.reciprocal(out=rnp1, in_=np1)
    f1 = pool.tile([LP, B], F32, tag="f1")
    nc.vector.tensor_mul(out=f1, in0=n2, in1=rs)
    fac = pool.tile([LP, B], F32, tag="fac")
    nc.vector.tensor_mul(out=fac, in0=f1, in1=rnp1)
    v = pool.tile([LP, BD], F32, tag="v")
    nc.vector.tensor_mul(
        out=_bd(v), in0=_bd(ss), in1=fac.unsqueeze(2).to_broadcast([LP, B, D])
    )
    return v


def _build_nc():
    nc = bass.Bass("TRN2", target_bir_lowering=False, debug=False)
    # resident W+pose, row-tiled: [128 = 4 bands x (16c+16pad), NG, 640]
    PW_d = nc.dram_tensor("PW", [128, NG, BD + LP], BF16, kind="ExternalInput")
    # pass-1 deep layouts: partition = (k8, c)
    PD_d = nc.dram_tensor("PD", [128, NDC, LP], BF16, kind="ExternalInput")
    WD_d = nc.dram_tensor("WD", [128, NDC, BD], BF16, kind="ExternalInput")
    out_d = nc.dram_tensor("vout", [LP, BD], F32, kind="ExternalOutput")
    PW = PW_d.ap()
    PD = PD_d.ap()
    WD = WD_d.ap()
    vout = out_d.ap()

    with _ChunkedDrainTileContext(nc) as tc:
        import contextlib

        with contextlib.ExitStack() as ctx:
            keep = ctx.enter_context(tc.tile_pool(name="keep", bufs=1))
            big = ctx.enter_context(tc.tile_pool(name="big", bufs=3))
            sm_pool = ctx.enter_context(tc.tile_pool(name="smp", bufs=3))
            small = ctx.enter_context(tc.tile_pool(name="small", bufs=2))

            eps_t = keep.tile([LP, 1], F32, tag="eps")
            nc.vector.memset(eps_t, EPS)

            # resident per-group W+pose tiles (one DMA each, reused 2x)
            pw_tiles = []
            for g in range(NG):
                t = keep.tile([128, BD + LP], BF16, tag=f"pw{g}")
                nc.sync.dma_start(out=t, in_=PW[:, g, :])
                pw_tiles.append(t)

            # ---------- pass 1: s1 = (1/32) sum_k V  (128-deep contraction) ----
            pd_t = keep.tile([128, NDC, LP], BF16, tag="pd")
            nc.sync.dma_start(out=pd_t, in_=PD)
            with tc.tile_pool(name="wd", bufs=2) as wd_pool, \
                 tc.tile_pool(name="s1", bufs=1, space="PSUM") as s1_pool:
                psum_s = s1_pool.tile([LP, BD], F32)
                WCH = 3
                for cb in range(NDC // WCH):
                    wd_t = wd_pool.tile([128, WCH, BD], BF16, tag="wd")
                    nc.sync.dma_start(
                        out=wd_t, in_=WD[:, cb * WCH:(cb + 1) * WCH, :])
                    for j in range(WCH):
                        ci = cb * WCH + j
                        nc.tensor.matmul(
                            psum_s,
                            lhsT=pd_t[:, ci, :],
                            rhs=wd_t[:, j, :],
                            start=(ci == 0),
                            stop=(ci == NDC - 1),
                        )
                ss1 = keep.tile([LP, BD], F32, tag="ss1")
                nc.scalar.mul(out=ss1, in_=psum_s, mul=1.0 / B)
            w = _squash(nc, small, ss1, eps_t)  # v1 f32; also w for iter 2

            vp_pool = ctx.enter_context(
                tc.tile_pool(name="vp", bufs=2, space="PSUM")
            )

            # ---------- iters 2..3 ----------
            for it in range(1, ITERS):
                s_acc = keep.tile([LP, BD], F32, tag=f"sacc{it}")
                nc.vector.memset(s_acc, 0.0)
                wbf = keep.tile([LP, B, D], BF16, tag=f"wbf{it}")
                nc.vector.tensor_copy(wbf, _bd(w))
                w_bcast = wbf.unsqueeze(1).to_broadcast([LP, G, B, D])

                # software-pipelined stages: per step t,
                #   scalar: exp(t-3), evac(t)   (exp first: its input is old)
                #   PE    : mm(t)
                #   DVE   : um/u-tree(t-2), Z/rZ/c(t-3), sg-tree/acc(t-5)
                #   gpsimd: sm(t-3)
                # every consumer's inputs are >= 1 full step old, so no
                # producer-chasing via subtile deps
                vt_q, u_q, e_q, c_q, sm_q, sg_q = {}, {}, {}, {}, {}, {}
                for t in range(NG + 6):
                    if 3 <= t < NG + 3:
                        g = t - 3
                        u = u_q.pop(g)
                        e = small.tile([LP, G, B], BF16, tag="e", bufs=3)
                        nc.scalar.activation(
                            out=e.rearrange("p g b -> p (g b)"),
                            in_=u.rearrange("p g b -> p (g b)"), func=ACT.Exp)
                        e_q[g] = e
                    if t < NG:
                        pwt = pw_tiles[t]
                        vp = vp_pool.tile([LP, G, BD], F32, tag="vp")
                        for j in range(G):
                            nc.tensor.matmul(
                                vp[:, j, :],
                                lhsT=pwt[32 * j:32 * j + 16, BD:BD + LP],
                                rhs=pwt[32 * j:32 * j + 16, 0:BD],
                                start=True,
                                stop=True,
                                tile_position=(32 * j, 0),
                            )
                        vt = big.tile([LP, G * BD], BF16, tag="vt", bufs=5,
                                      padded_shape=[LP, G * BD + 512])
                        nc.scalar.copy(
                            out=vt, in_=vp.rearrange("p g bd -> p (g bd)")
                        )
                        vt_q[t] = vt
                    if 2 <= t < NG + 2:
                        g = t - 2
                        vt = vt_q[g]
                        um = big.tile([LP, G + 1, B, D], BF16, tag="um", bufs=3, name="um")[:, 0:G]
                        nc.vector.tensor_mul(
                            out=um,
                            in0=vt.rearrange("p (g b d) -> p g b d", b=B, d=D),
                            in1=w_bcast,
                        )
                        u8 = small.tile([LP, G + 1, B, 8], BF16, tag="u8", name="u8")[:, 0:G]
                        nc.vector.tensor_add(
                            out=u8, in0=um[:, :, :, 0:8], in1=um[:, :, :, 8:16])
                        u4 = small.tile([LP, G, B, 4], BF16, tag="u4")
                        nc.vector.tensor_add(
                            out=u4, in0=u8[:, :, :, 0:4], in1=u8[:, :, :, 4:8])
                        u2 = small.tile([LP, G, B, 2], BF16, tag="u2")
                        nc.vector.tensor_add(
                            out=u2, in0=u4[:, :, :, 0:2], in1=u4[:, :, :, 2:4])
                        u = small.tile([LP, G, B], BF16, tag="u", bufs=3)
                        with nc.allow_low_precision("logits, feeds softmax"):
                            nc.vector.tensor_add(
                                out=u, in0=u2[:, :, :, 0], in1=u2[:, :, :, 1])
                        u_q[g] = u
                    if 3 <= t < NG + 3:
                        g = t - 3
                        e = e_q.pop(g)
                        Z = small.tile([LP, G], F32, tag="Z")
                        nc.vector.tensor_reduce(
                            out=Z, in_=e, axis=AX.X, op=OP.add,
                        )
                        rZ = small.tile([LP, G], F32, tag="rZ")
                        nc.vector.reciprocal(out=rZ, in_=Z)
                        c = small.tile([LP, G, B], BF16, tag="c", bufs=3)
                        nc.vector.tensor_mul(
                            out=c, in0=e,
                            in1=rZ.unsqueeze(2).to_broadcast([LP, G, B]),
                        )
                        sm = sm_pool.tile([LP, G + 1, B, D], BF16, tag="sm", bufs=3, name="sm")[:, 0:G]
                        nc.gpsimd.tensor_mul(
                            out=sm, in0=_bd(vt_q.pop(g)),
                            in1=c.unsqueeze(3).to_broadcast([LP, G, B, D]))
                        sm_q[g] = sm
                    if 5 <= t < NG + 5:
                        g = t - 5
                        sm = sm_q.pop(g)
                        sgA = small.tile([LP, B, D], BF16, tag="sgA")
                        nc.vector.tensor_add(
                            out=sgA, in0=sm[:, 0, :, :], in1=sm[:, 2, :, :])
                        sgB = big.tile([LP, B, D], BF16, tag="sgB")
                        nc.vector.tensor_add(
                            out=sgB, in0=sm[:, 1, :, :], in1=sm[:, 3, :, :])
                        if g % 2 == 0:
                            sg1 = small.tile([LP, B, D], BF16, tag="sg1a")
                        else:
                            sg1 = big.tile([LP, B, D], BF16, tag="sg1b")
                        nc.vector.tensor_add(out=sg1, in0=sgA, in1=sgB)
                        sg_q[g] = sg1
                        if g % 2 == 1:
                            sgp = sm_pool.tile([LP, B, D], BF16, tag="sgp")
                            nc.vector.tensor_add(
                                out=sgp, in0=sg_q.pop(g - 1), in1=sg_q.pop(g))
                            nc.vector.tensor_add(
                                out=s_acc, in0=s_acc,
                                in1=sgp.rearrange("p b d -> p (b d)"))

                v_it = _squash(nc, small, s_acc, eps_t)
                if it < ITERS - 1:
                    w_new = keep.tile([LP, BD], F32, tag=f"w{it}")
                    nc.vector.tensor_add(out=w_new, in0=w, in1=v_it)
                    w = w_new
                else:
                    nc.sync.dma_start(out=vout[:, :], in_=v_it)
    _split_excess_waits(nc)
    return nc


def _host_prep(pose, W):
    """unfold + shard + build the three device layouts per core."""
    pose = np.asarray(pose, dtype=np.float32)
    W = np.asarray(W, dtype=np.float32)
    b = pose.shape[0]
    cols = np.empty((b, A * C, KK, OH, OW), dtype=np.float32)
    for ki in range(K):
        for kj in range(K):
            cols[:, :, ki * K + kj] = pose[
                :, :, ki:ki + STRIDE * (OH - 1) + 1:STRIDE,
                kj:kj + STRIDE * (OW - 1) + 1:STRIDE,
            ]
    # (b, A, C, KK, l) -> (b, l, KK, A, C) -> (npos, KKA, C)
    p = cols.reshape(b, A, C, KK, L).transpose(0, 4, 3, 1, 2).reshape(
        NPOS, KKA, C
    )
    p_pad = np.zeros((NPOS_PAD, KKA, C), dtype=np.float32)
    p_pad[:NPOS] = p

    # W layouts (shared across cores)
    W_bf = W.astype(BF)
    # row-tiled: PW_w[32j+c, g, bd] = W[4g+j, bd, c]
    PW_w = np.zeros((128, NG, BD), dtype=BF)
    Wr = W_bf.reshape(NG, G, BD, C)
    for j in range(G):
        PW_w[32 * j:32 * j + 16, :, :] = Wr[:, j, :, :].transpose(2, 0, 1)
    # deep: WD[16u+c, ci, bd] = W[8ci+u, bd, c]
    WD_np = np.empty((128, NDC, BD), dtype=BF)
    Wd = W_bf.reshape(NDC, KD, BD, C)
    for u in range(KD):
        WD_np[16 * u:16 * u + 16, :, :] = Wd[:, u, :, :].transpose(2, 0, 1)

    in_maps = []
    for i in range(NCORES):
        sh = p_pad[i * LP:(i + 1) * LP].astype(BF)     # [LP, KKA, C]
        # row-tiled pose: PW_p[32j+c, g, l] = p[l, 4g+j, c]
        PW = np.zeros((128, NG, BD + LP), dtype=BF)
        PW[:, :, :BD] = PW_w
        pr = sh.reshape(LP, NG, G, C)
        for j in range(G):
            PW[32 * j:32 * j + 16, :, BD:] = pr[:, :, j, :].transpose(2, 1, 0)
        # deep pose: PD[16u+c, ci, l] = p[l, 8ci+u, c]
        PD = np.empty((128, NDC, LP), dtype=BF)
        pd = sh.reshape(LP, NDC, KD, C)
        for u in range(KD):
            PD[16 * u:16 * u + 16, :, :] = pd[:, :, u, :].transpose(2, 1, 0)
        in_maps.append({"PW": PW, "PD": PD, "WD": WD_np})
    return in_maps


def _gather(results):
    v = np.concatenate([r["vout"] for r in results], axis=0)  # [1024, 512]
    v = v[:NPOS].reshape(NB, L, BD).transpose(0, 2, 1)
    return np.ascontiguousarray(v.reshape(NB, BD, OH, OW), dtype=np.float32)


def _split_excess_waits(nc, max_waits=1):
    """walrus (CoreV2/V3) accepts at most 2 sync-wait commands per
    compute instruction and 1 per DMA; hoist excess waits onto NOPs
    just before, same engine."""
    n_split = 0
    for f in nc.m.functions:
        for bb in f.blocks:
            il = bb.instructions
            out = []
            changed = False
            for inst in il:
                lim = max_waits
                si = inst.sync_info
                if si is not None and si.on_wait and len(si.on_wait) > lim:
                    waits = list(si.on_wait)
                    excess, kept = waits[:-lim], waits[-lim:]
                    for i in range(0, len(excess), max_waits):
                        nop = mybir.InstNoOp(
                            name=f"{inst.name}-w{i}",
                            sync_info=mybir.SyncInfo(
                                on_wait=excess[i:i + max_waits], on_update=[]
                            ),
                            bass_nofuse=True,
                            engine=inst.engine,
                        )
                        out.append(nop)
                        n_split += 1
                    inst.sync_info = mybir.SyncInfo(
                        on_wait=kept, on_update=list(si.on_update or [])
                    )
                    changed = True
                out.append(inst)
            if changed:
                bb.instructions = out
    return n_split


_NC_CACHE = {}


def _get_nc(mm_dtype=None):
    key = "v2"
    if key not in _NC_CACHE:
        _NC_CACHE[key] = _build_nc()
    return _NC_CACHE[key]


def _run(pose, W, trace=False, mm_dtype=None):
    nc = _get_nc()
    in_maps = _host_prep(pose, W)
    res = run_bass_kernel_spmd(
        nc, in_maps, core_ids=list(range(NCORES)), trace=trace
    )
    return _gather(res.results), res


def kernel(pose, W):
    out, _ = _run(pose, W)
    return out
